# revision 36
# baseline (speedup 1.0000x reference)
"""Trainium2 Bass kernel: LoRA multi-head attention with decomposed (SAM-style)
relative position bias, sharded across 8 NeuronCores.

Shapes (hardcoded): x (1,64,64,768), 12 heads x 64 dims, n=4096 tokens,
rank-4 LoRA on q/v, rel_h/rel_w (127,64).

Strategy (two SPMD launches, no collectives):
  Launch A (token-sharded): core c computes qkv^T (2304 x 512) for its 512
    tokens. LoRA is folded into W_qkv on the host (W_eff = W + B_s @ A), so
    A is a pure GEMM. b_q fused via ACT bias; b_k dropped (softmax-
    invariant); b_v folded into b_proj on host.
  Host: reassemble q^T/k^T/v^T, build augmented operands (below).
  Launch B (query-sharded): core c computes attention + projection for its
    512 queries over all 12 heads.

Launch B engine split (keys ordered kw-fast, chunked 128 = 2 kh-rows):
  - bias_h rides the QK^T matmul: stationary Ka[h,c] = [K_h^T chunk ;
    one-hot-over-kh block], moving Qa^T = [0.125*q^T ; bh^T] where
    bh[q,kh] = q . Rh[i(q),kh].
  - bias_w is applied multiplicatively AFTER the exp (exp(s0+bw) =
    exp(s0)*exp(bw)): DVE bf16 multiply with a host-precomputed
    exp(bw) tile (replicated to 128 partitions and tiled x3 groups).
  - exp on ACT in [128,1536] PSUM groups (3 score chunks per ACT) to
    amortize the ~352-cycle per-instruction overhead.
  - softmax rowsum = ones column appended to V (M=65); per-head
    normalization chain (copy rowsum -> reciprocal_approx_fast ->
    partition_broadcast on GPSIMD -> multiply) runs entirely off the PE
    queue so the PE never stalls mid-stream (keeps HAM at 2.4 GHz).
  - projection packs head pairs: contraction 128 = 2 heads x 64 ch.
PSUM layout per core in B: scores 2 x [128,1536] (banks 0-5),
AV accumulators 2 x [128,512] (banks 6-7).
All matmul operands are bf16 (fp32 PSUM accumulation).
"""

import os
import sys

import ml_dtypes
import numpy as np

sys.path.insert(0, "/opt/trn_rl_repo")

BF = ml_dtypes.bfloat16


def _bf(a):
    return np.ascontiguousarray(a).astype(BF)

import concourse.bass as bass  # noqa: E402
import concourse.tile as tile  # noqa: E402
from concourse import bacc, mybir  # noqa: E402

DT = mybir.dt
F32 = DT.float32
BF16 = DT.bfloat16
AF = mybir.ActivationFunctionType

DIM = 768
NH = 12
HD = 64
HW = 64  # h == w == 64
N = HW * HW  # 4096 tokens
RANK = 4
LORA_SCALING = 1.0 / RANK
SCALE = HD ** -0.5
NCORES = 8
TPC = N // NCORES  # 512 tokens/queries per core
ROWS_PC = TPC // HW  # 8 grid rows per core
NKC = N // 128  # 32 key chunks
NIC = DIM // 128  # 6 input-channel chunks
NOC = 3 * DIM // 128  # 18 qkv output chunks

# launch B score-group pattern: 3-chunk groups (=[128,1536] psum), last is 2
GROUPS = [2] + [3] * 10
assert sum(GROUPS) == NKC


def _new_nc() -> bacc.Bacc:
    return bacc.Bacc("TRN2", target_bir_lowering=False, debug=False)


def build_launch_a() -> bass.Bass:
    nc = _new_nc()
    # x^T rearranged on host to [128, NIC, TPC]
    xt_d = nc.declare_dram_parameter("XTR", [128, NIC, TPC], BF16, isOutput=False)
    # W_eff^T rearranged on host to [128, NIC, 3*DIM]
    wt_d = nc.declare_dram_parameter("WTR", [128, NIC, 3 * DIM], BF16, isOutput=False)
    bq_d = nc.declare_dram_parameter("BQB", [128, NIC], F32, isOutput=False)
    # [p, oc, t] layout; host transposes back to (2304, TPC)
    out_d = nc.declare_dram_parameter("QKVT", [128, NOC, TPC], BF16, isOutput=True)

    OCG = 3  # output chunks per weight-DMA / out-DMA group

    with tile.TileContext(nc) as tc:
        with (
            nc.allow_low_precision(reason="bf16 matmul operands are intended"),
            tc.tile_pool(name="cst", bufs=1) as cst,
            tc.tile_pool(name="wt", bufs=4) as wt_p,
            tc.tile_pool(name="sb", bufs=3) as sb,
            tc.tile_pool(name="ps", bufs=4, space=bass.MemorySpace.PSUM) as ps,
        ):
            xt = cst.tile([128, NIC * TPC], BF16, tag="xt")
            nc.sync.dma_start(xt[:], xt_d[:])
            bq_t = cst.tile([128, NIC], F32, tag="bq")
            nc.scalar.dma_start(bq_t[:], bq_d[:])

            # warm the PE clock (HAM) with dummy matmuls while DMAs land
            wu = cst.tile([128, 128], BF16, tag="wu")
            nc.vector.memset(wu[:], 0.0)
            wups = ps.tile([128, 128], F32, tag="wups", name="wups")
            for _ in range(28):
                nc.tensor.matmul(wups[:], wu[:], wu[:], start=True, stop=True)

            for og in range(NOC // OCG):
                w = wt_p.tile([128, NIC * OCG * 128], BF16, tag="wt")
                nc.sync.dma_start(
                    w[:], wt_d[:, :, og * OCG * 128:(og + 1) * OCG * 128]
                )
                outs = sb.tile([128, OCG * TPC], BF16, tag="out_s")
                for j in range(OCG):
                    oc = og * OCG + j
                    app = ps.tile([128, TPC], F32, tag="qkv_ps")
                    for ic in range(NIC):
                        nc.tensor.matmul(
                            app[:],
                            (w[:, (ic * OCG + j) * 128:(ic * OCG + j + 1) * 128]),
                            (xt[:, ic * TPC:(ic + 1) * TPC]),
                            start=(ic == 0),
                            stop=(ic == NIC - 1),
                        )
                    osl = outs[:, j * TPC:(j + 1) * TPC]
                    if oc < NIC:
                        # q gets b_q added during the PSUM->SBUF copy (ACT)
                        nc.scalar.activation(
                            osl, app[:], AF.Identity,
                            bias=bq_t[:, oc:oc + 1], scale=1.0
                        )
                    else:
                        # k/v: plain copy on DVE (keeps ACT free)
                        nc.vector.tensor_copy(osl, app[:])
                # output DMAs issue from the (mostly idle) ACT queue so the
                # sync queue stays dedicated to the weight feed
                nc.scalar.dma_start(
                    out_d[:, og * OCG:(og + 1) * OCG, :], outs[:]
                )
    nc.compile()
    return nc


def build_launch_b() -> bass.Bass:
    nc = _new_nc()
    # qa ([:, :TPC]) and exp(bw) tiled x3 ([:, TPC:]) merged into one tensor
    qe_d = nc.declare_dram_parameter("QE", [NH, 128, 4 * TPC], BF16, isOutput=False)
    # ka ([:, :NKC*128]) and va ([:, NKC*128:]) merged into one tensor
    kv_d = nc.declare_dram_parameter("KV", [NH, 128, 2 * NKC * 128], BF16,
                                     isOutput=False)
    # W_proj^T packed by head pairs: [NH//2, 128, DIM]
    wpt_d = nc.declare_dram_parameter("WPT", [NH // 2, 128, DIM], BF16, isOutput=False)
    bp_d = nc.declare_dram_parameter("BP", [128, NIC], F32, isOutput=False)
    # [p, oc, t] layout; host transposes back to (768, TPC)
    out_d = nc.declare_dram_parameter("OUTT", [128, NIC, TPC], F32, isOutput=True)

    KAW = NKC * 128  # 4096

    with tile.TileContext(nc) as tc:
        with (
            nc.allow_low_precision(reason="bf16 matmul operands are intended"),
            tc.tile_pool(name="cst", bufs=1) as cst,
            tc.tile_pool(name="qe", bufs=3) as qe_p,
            tc.tile_pool(name="kv", bufs=3) as kv_p,
            tc.tile_pool(name="at", bufs=3) as at_p,
            tc.tile_pool(name="atb", bufs=3) as atb_p,
            tc.tile_pool(name="nrm", bufs=2) as nrm_p,
            tc.tile_pool(name="per_head", bufs=1) as ph,
            tc.tile_pool(name="sps", bufs=2, space=bass.MemorySpace.PSUM) as sps,
            tc.tile_pool(name="aps", bufs=2, space=bass.MemorySpace.PSUM) as aps,
        ):
            # attention outputs, packed per head pair for the projection
            att_n = [ph.tile([128, TPC], BF16, tag=f"attn{hp}", name=f"attn{hp}")
                     for hp in range(NH // 2)]

            def head_inputs(h, chunked):
                qe = qe_p.tile([128, 4 * TPC], BF16, tag="qe")
                kv = kv_p.tile([128, 2 * KAW], BF16, tag="kv")
                if chunked:  # head 0: split so the first matmuls start early
                    nc.sync.dma_start(qe[:, 0:TPC], qe_d[h][:, 0:TPC])
                    nc.sync.dma_start(kv[:, 0:512], kv_d[h][:, 0:512])
                    nc.sync.dma_start(qe[:, TPC:], qe_d[h][:, TPC:])
                    nc.sync.dma_start(kv[:, 512:KAW], kv_d[h][:, 512:KAW])
                    nc.sync.dma_start(
                        kv[:, KAW:KAW + 512], kv_d[h][:, KAW:KAW + 512]
                    )
                    nc.sync.dma_start(kv[:, KAW + 512:], kv_d[h][:, KAW + 512:])
                else:
                    nc.sync.dma_start(qe[:], qe_d[h])
                    nc.sync.dma_start(kv[:], kv_d[h])
                return qe, kv

            # normalization chain for head h (entirely off the PE queue).
            # The rowsum row is reshaped to [64,8] by a tiny sync-queue DMA
            # so the DVE reciprocal runs 64-lane-parallel (~0.2us, not 4us).
            def norm_head_a(h, av_ps):
                rs = nrm_p.tile([1, TPC], F32, tag="rs", name=f"rs{h}")
                nc.vector.tensor_copy(rs[:], av_ps[HD:HD + 1, :])
                rcp = nrm_p.tile([1, TPC], F32, tag="rcp", name=f"rcp{h}")
                nc.vector.reciprocal_approx_fast(rcp[:], rs[:])
                bcs = nrm_p.tile([HD, TPC], F32, tag="bcs", name=f"bcs{h}")
                nc.gpsimd.partition_broadcast(bcs[:], rcp[:])
                return bcs

            def norm_head_b(h, av_ps, bcs):
                hp, half = h // 2, h % 2
                nc.vector.tensor_mul(
                    att_n[hp][half * HD:(half + 1) * HD, :],
                    av_ps[0:HD, :],
                    bcs[:],
                )

            # warm the PE clock (HAM) with dummy matmuls while DMAs land
            wu = cst.tile([128, 128], BF16, tag="wu")
            nc.vector.memset(wu[:], 0.0)
            wups = aps.tile([128, TPC], F32, tag="av", name="wups")
            for _ in range(12):
                nc.tensor.matmul(wups[:, 0:128], wu[:], wu[:], start=True, stop=True)

            pending = None  # (h, av_ps) awaiting normalization
            nexth = [head_inputs(0, chunked=True)]
            for h in range(NH):
                qe, kv = nexth.pop()
                qa = qe[:, 0:TPC]
                ebw = qe[:, TPC:]
                ka = kv[:, 0:KAW]
                va = kv[:, KAW:]
                if h + 1 < NH:  # prefetch next head's inputs
                    nexth.append(head_inputs(h + 1, chunked=False))
                if h == 1:  # projection constants, needed only at the end
                    bp_t = cst.tile([128, NIC], F32, tag="bp")
                    nc.sync.dma_start(bp_t[:], bp_d[:])
                    wpt = []
                    for hp in range(NH // 2):
                        w = cst.tile([128, DIM], BF16, tag=f"wpt{hp}")
                        nc.sync.dma_start(w[:], wpt_d[hp])
                        wpt.append(w)
                av_ps = aps.tile([128, TPC], F32, tag="av")
                c0 = 0
                bcs_prev = None
                for gi, gsz in enumerate(GROUPS):
                    gw = gsz * TPC
                    s = sps.tile([128, 3 * TPC], F32, tag="scores")
                    for u in range(gsz):
                        c = c0 + u
                        nc.tensor.matmul(
                            s[:, u * TPC:(u + 1) * TPC],
                            (ka[:, c * 128:(c + 1) * 128]),
                            (qa[:]),
                            start=True, stop=True,
                        )
                    # normalization of the previous head slots in over the
                    # first groups: DVE/GPSIMD/DMA work, no PE stall
                    if gi == 0 and pending is not None:
                        bcs_prev = norm_head_a(*pending)
                    if gi == 2 and pending is not None:
                        norm_head_b(*pending, bcs_prev)
                        pending = None
                    at = at_p.tile([128, 3 * TPC], BF16, tag="at")
                    nc.scalar.activation(at[:, 0:gw], s[:, 0:gw], AF.Exp)
                    atb = atb_p.tile([128, 3 * TPC], BF16, tag="atb")
                    nc.vector.tensor_mul(atb[:, 0:gw], at[:, 0:gw], ebw[:, 0:gw])
                    for u in range(gsz):
                        c = c0 + u
                        nc.tensor.matmul(
                            av_ps[:],
                            (va[:, c * 128:(c + 1) * 128]),
                            (atb[:, u * TPC:(u + 1) * TPC]),
                            start=(c == 0), stop=(c == NKC - 1),
                        )
                    c0 += gsz
                pending = (h, av_ps)

            # final-head normalization races the projection: the hp=0..4
            # partial sums need only heads 0..9, so only the last 6 matmuls
            # wait on head 11's norm chain.
            bcs_last = norm_head_a(*pending)
            pj3 = [sps.tile([128, 3 * TPC], F32, tag="scores", name=f"pj{t}")
                   for t in range(2)]
            for t in range(2):
                for hp in range(NH // 2 - 1):
                    for j in range(3):
                        nc.tensor.matmul(
                            pj3[t][:, j * TPC:(j + 1) * TPC],
                            (wpt[hp][:, (t * 3 + j) * 128:(t * 3 + j + 1) * 128]),
                            (att_n[hp][:]),
                            start=(hp == 0), stop=False,
                        )
            norm_head_b(*pending, bcs_last)
            hp = NH // 2 - 1
            for t in range(2):
                for j in range(3):
                    oc = t * 3 + j
                    nc.tensor.matmul(
                        pj3[t][:, j * TPC:(j + 1) * TPC],
                        (wpt[hp][:, oc * 128:(oc + 1) * 128]),
                        (att_n[hp][:]),
                        start=False, stop=True,
                    )
            for t in range(2):
                outs = nrm_p.tile([128, 3 * TPC], F32, tag="out_s")
                for j in range(3):
                    oc = t * 3 + j
                    # +b_proj during the PSUM->SBUF copy, on the idle DVE
                    nc.vector.tensor_scalar_add(
                        outs[:, j * TPC:(j + 1) * TPC],
                        pj3[t][:, j * TPC:(j + 1) * TPC],
                        bp_t[:, oc:oc + 1],
                    )
                nc.sync.dma_start(out_d[:, t * 3:(t + 1) * 3, :], outs[:])
    nc.compile()
    return nc


_CACHE: dict = {}


def _programs():
    if "A" not in _CACHE:
        _CACHE["A"] = build_launch_a()
        _CACHE["B"] = build_launch_b()
    return _CACHE["A"], _CACHE["B"]


def _host_prep_a(x, W_qkv, A_q, B_q, A_v, B_v, b_qkv):
    xf = x.reshape(N, DIM).T  # (768, 4096)
    # fold LoRA into the qkv weight (exact: x@A.T@B.T*s == x@(B_s@A).T)
    W_eff = W_qkv.astype(np.float64).copy()
    W_eff[:DIM] += (B_q.astype(np.float64) * LORA_SCALING) @ A_q.astype(np.float64)
    W_eff[2 * DIM:] += (B_v.astype(np.float64) * LORA_SCALING) @ A_v.astype(np.float64)
    wt = W_eff.T.astype(np.float32)  # (768, 2304)
    wtr = wt.reshape(NIC, 128, 3 * DIM).transpose(1, 0, 2)  # (128, 6, 2304)
    shared = {
        "WTR": _bf(wtr),
        "BQB": np.ascontiguousarray(
            b_qkv[:DIM].reshape(NIC, 128).T
        ).astype(np.float32),
    }
    in_maps = []
    for c in range(NCORES):
        m = dict(shared)
        xc = xf[:, c * TPC:(c + 1) * TPC]  # (768, 512)
        m["XTR"] = _bf(xc.reshape(NIC, 128, TPC).transpose(1, 0, 2))
        in_maps.append(m)
    return in_maps


def _get_rel(size, rel_pos):
    coords = np.arange(size)[:, None] - np.arange(size)[None, :] + (size - 1)
    return rel_pos[coords]  # (size, size, hd)


def _host_prep_b(qT, kT, vT, rel_h, rel_w, W_proj, b_proj, b_v):
    # shared (same for all cores)
    ka = np.zeros((NH, NKC, 128, 128), np.float32)
    ka[:, :, :HD, :] = kT.reshape(NH, HD, NKC, 128).transpose(0, 2, 1, 3)
    for ck in range(NKC):
        ka[:, ck, HD + 2 * ck, 0:HD] = 1.0
        ka[:, ck, HD + 2 * ck + 1, HD:128] = 1.0
    va = np.zeros((NH, NKC, 128, 128), np.float32)
    va[:, :, :, :HD] = vT.reshape(NH, HD, NKC, 128).transpose(0, 2, 3, 1)
    va[:, :, :, HD] = 1.0
    wpt = np.ascontiguousarray(
        W_proj.T.reshape(NH // 2, 128, DIM)
    )
    bp = np.ascontiguousarray(
        (b_proj + W_proj @ b_v).astype(np.float32).reshape(NIC, 128).T
    )
    Rh = _get_rel(HW, rel_h)  # (64 i, 64 kh, 64 ch)
    Rw = _get_rel(HW, rel_w)  # (64 j, 64 kw, 64 ch)

    kab = ka.transpose(0, 2, 1, 3).reshape(NH, 128, NKC * 128)
    vab = va.transpose(0, 2, 1, 3).reshape(NH, 128, NKC * 128)
    kv = np.concatenate([kab, vab], axis=2)  # (NH, 128, 2*4096)
    shared = {
        "KV": _bf(kv),
        "WPT": _bf(wpt), "BP": bp,
    }
    in_maps = []
    for c in range(NCORES):
        q_c = qT[:, c * TPC:(c + 1) * TPC]  # (768, 512)
        qr = q_c.reshape(NH, HD, ROWS_PC, HW)  # h, ch, row, j
        rh_c = Rh[c * ROWS_PC:(c + 1) * ROWS_PC]  # (8, kh, ch)
        bh = np.einsum("hcrj,rkc->hkrj", qr, rh_c, optimize=True)
        bw = np.einsum("hcrj,jkc->hkrj", qr, Rw, optimize=True)
        qe = np.empty((NH, 128, 4 * TPC), np.float32)
        qe[:, :HD, 0:TPC] = SCALE * q_c.reshape(NH, HD, TPC)
        qe[:, HD:, 0:TPC] = bh.reshape(NH, HD, TPC)
        # exp(bw), replicated on both partition halves, tiled x3 groups
        ebw1 = np.exp(bw.reshape(NH, HD, TPC))
        for rep in range(3):
            sl = slice((1 + rep) * TPC, (2 + rep) * TPC)
            qe[:, :HD, sl] = ebw1
            qe[:, HD:, sl] = ebw1
        m = dict(shared)
        m["QE"] = _bf(qe)
        in_maps.append(m)
    return in_maps


def _run_spmd(nc, in_maps, trace=False):
    from concourse import bass_utils

    cores = list(range(NCORES))
    if trace:
        # artifact upload needs a bucket this sandbox doesn't have
        bass_utils.upload_artifacts = lambda d: str(d)
        try:
            return bass_utils.run_bass_kernel_spmd(nc, in_maps, cores, trace=True)
        except Exception as e:  # fall back to an untraced run
            print(f"traced run failed ({type(e).__name__}: {e})", file=sys.stderr)
    return bass_utils.run_bass_kernel_spmd(nc, in_maps, cores, trace=False)


def kernel(
    x, W_qkv, b_qkv, A_q, B_q, A_v, B_v, rel_h, rel_w, W_proj, b_proj,
    _collect_times=None,
):
    x = np.asarray(x, np.float32)
    W_qkv = np.asarray(W_qkv, np.float32)
    b_qkv = np.asarray(b_qkv, np.float32)
    A_q = np.asarray(A_q, np.float32)
    B_q = np.asarray(B_q, np.float32)
    A_v = np.asarray(A_v, np.float32)
    rel_h = np.asarray(rel_h, np.float32)
    rel_w = np.asarray(rel_w, np.float32)
    B_v = np.asarray(B_v, np.float32)
    W_proj = np.asarray(W_proj, np.float32)
    b_proj = np.asarray(b_proj, np.float32)

    nc_a, nc_b = _programs()
    trace = _collect_times is not None

    maps_a = _host_prep_a(x, W_qkv, A_q, B_q, A_v, B_v, b_qkv)
    res_a = _run_spmd(nc_a, maps_a, trace=trace)
    # per-core result is [128, NOC, TPC]; transpose back to (2304, TPC)
    qkvT = np.concatenate(
        [r["QKVT"].transpose(1, 0, 2).reshape(3 * DIM, TPC)
         for r in res_a.results],
        axis=1,
    ).astype(np.float32)  # (2304, 4096)
    qT, kT, vT = qkvT[:DIM], qkvT[DIM:2 * DIM], qkvT[2 * DIM:]

    maps_b = _host_prep_b(
        qT, kT, vT, rel_h, rel_w, W_proj, b_proj, b_qkv[2 * DIM:]
    )
    res_b = _run_spmd(nc_b, maps_b, trace=trace)
    outT = np.concatenate(
        [r["OUTT"].transpose(1, 0, 2).reshape(DIM, TPC) for r in res_b.results],
        axis=1,
    )  # (768, 4096)
    if _collect_times is not None:
        _collect_times.append((res_a.exec_time_ns, res_b.exec_time_ns))
    return np.ascontiguousarray(outT.T).reshape(1, HW, HW, DIM)
